# revision 42
# baseline (speedup 1.0000x reference)
"""Trainium2 Bass kernel for dual-score multi-head self-attention.

Reference computation (per batch b):
    q   = x @ Wq.T + bq          k = x @ Wk.T + bk
    v   = x @ Wv.T + bv          pos = pos_emb @ Wp.T + bp
    scores = (q k^T + q pos^T) / sqrt(dh)   (per head)
    out = softmax(scores) @ v, merged heads, @ Wo.T + bo

Algebraic folds:
  * q k^T + q pos^T == q (k+pos)^T; k+pos accumulates in one PSUM tile.
  * v's bias commutes through softmax (rows sum to 1): bo' = bo + Wo @ bv.
  * softmax denominators ride along in the attention*V matmul via ones
    columns appended to V (even heads [v|ones], odd heads [ones|v] so
    every DVE op in the normalization runs at a legal base partition).

Sharding: batch parallel, one batch per core, weights replicated, no
collectives. See bottom for host-side prep.
"""

import numpy as np

try:
    import concourse.bass as bass  # noqa: F401
except ImportError:  # pragma: no cover - container default path
    import sys

    for p in ("/opt/trn_rl_repo", "/root/.axon_site/_ro/trn_rl_repo"):
        if p not in sys.path:
            sys.path.insert(0, p)

import ml_dtypes

import concourse.bass as bass
import concourse.mybir as mybir
import concourse.tile as tile
from concourse import bacc
from concourse.bass import ts
from concourse.bass_utils import run_bass_kernel_spmd

P = 128
T = 2048
D = 512
H = 8
DH = 64
B = 8
MT = D // P  # 4 feature tiles
KT = T // P  # 16 kpos tiles
QC = 512  # q chunk (one PSUM bank of f32)
NQC = T // QC

BF = mybir.dt.bfloat16
F32 = mybir.dt.float32
U8 = mybir.dt.uint8
FP8 = mybir.dt.float8e4
ADD = mybir.AluOpType.add
MULT = mybir.AluOpType.mult
EXP = mybir.ActivationFunctionType.Exp
COPY = mybir.ActivationFunctionType.Copy
IDENT = mybir.ActivationFunctionType.Identity
DR = mybir.MatmulPerfMode.DoubleRow

N_CORES = 8
KTP = KT // 2  # 8 key-tile PAIRS (fp8 DoubleRow granularity for attn*V)

# kt indices whose exp tile is computed on VectorE via the Schraudolph
# fp8-bit trick instead of ScalarE (load balancing; [] = all on ScalarE).
# All odd kts: each kt pair's two exps run concurrently on both engines
# and neither engine ever faces back-to-back exp tiles.
SCHRAUDOLPH_KTS = (1, 3, 5, 7, 9, 11, 13, 15)
AV_TRAIL = 2  # AV matmul pairs trail the score matmuls by this many ktp slots
# e4m3 bits(exp(s/8)) ~= round(A*s + B): 8 codes/octave, bias 7.
_SCH_A = 8.0 * 1.4426950408889634 * 0.125
_SCH_B = 8.0 * (7.0 - 0.043677448)


def _emit(ctx, tc, io, dbg=None):
    nc = tc.nc

    # ---- persistent SBUF tensors -------------------------------------
    const_pool = ctx.enter_context(tc.tile_pool(name="const", bufs=1))

    def single(name, shape, dtype):
        return const_pool.tile(shape, dtype, name=name, tag=name)

    w_sb = {w: single(f"w_{w}", [P, MT, D], BF) for w in
            ("wq", "wk", "wp", "wv", "wo")}
    b_sb = {b: single(f"b_{b}", [P, MT], F32) for b in ("bq", "bob")}
    xT_sb = single("xT_sb", [P, MT, T], BF)
    posT_sb = single("posT_sb", [P, MT, T], BF)
    qT_sb = [single(f"qT{m}", [P, T], BF) for m in range(MT)]
    kT_sb = [single(f"kT{m}", [P, T], BF) for m in range(MT)]
    ctx_sb = [single(f"ctxT{m}", [P, T], BF) for m in range(MT)]
    # v for kt pair ktp: [:, i, h, :] = v(+ones) of head h, key tile 2*ktp+i
    v_sb = [single(f"v{ktp}", [P, 2, H, P], FP8) for ktp in range(KTP)]

    # ones-regions of v on GpSimd while the sync queue streams the
    # critical loads; first AV needs them well before GpSimd's bulk DMAs.
    for ktp in range(KTP):
        nc.gpsimd.memset(v_sb[ktp][:, :, 0 : H : 2, DH:P], 1.0)
        nc.gpsimd.memset(v_sb[ktp][:, :, 1 : H : 2, 0:DH], 1.0)

    # ---- input DMAs -------------------------------------------------
    # The first q-projection matmul only needs wq[k] + x[k] chunk0: feed
    # those as interleaved per-k transfers so PE starts ~2us in. All other
    # tensors arrive as one large DMA each (queue slots are ~0.7us apiece).
    # Sync carries the critical path; GpSimd carries the late-need bulk.
    for k in range(MT):
        nc.sync.dma_start(out=w_sb["wq"][:, k, :], in_=io["wqT"][:, k, :])
        nc.sync.dma_start(out=xT_sb[:, k, 0:QC], in_=io["xT"][:, k, 0:QC])
    nc.sync.dma_start(out=w_sb["wk"], in_=io["wkT"])
    nc.sync.dma_start(out=w_sb["wp"], in_=io["wpT"])
    nc.sync.dma_start(out=posT_sb[:, :, 0:QC], in_=io["posT"][:, :, 0:QC])
    nc.sync.dma_start(out=w_sb["wv"], in_=io["wvT"])
    nc.sync.dma_start(out=w_sb["wo"], in_=io["woT"])
    nc.gpsimd.dma_start(out=b_sb["bq"], in_=io["bq"])
    nc.gpsimd.dma_start(out=xT_sb[:, :, QC : 2 * QC], in_=io["xT"][:, :, QC : 2 * QC])
    nc.gpsimd.dma_start(
        out=posT_sb[:, :, QC : 2 * QC], in_=io["posT"][:, :, QC : 2 * QC]
    )
    nc.gpsimd.dma_start(
        out=xT_sb[:, :, 2 * QC : 3 * QC], in_=io["xT"][:, :, 2 * QC : 3 * QC]
    )
    nc.gpsimd.dma_start(
        out=posT_sb[:, :, 2 * QC : 3 * QC], in_=io["posT"][:, :, 2 * QC : 3 * QC]
    )
    nc.gpsimd.dma_start(out=xT_sb[:, :, 3 * QC : T], in_=io["xT"][:, :, 3 * QC : T])
    nc.gpsimd.dma_start(
        out=posT_sb[:, :, 3 * QC : T], in_=io["posT"][:, :, 3 * QC : T]
    )
    nc.gpsimd.dma_start(out=b_sb["bob"], in_=io["bob"])

    # ---- pools --------------------------------------------------------
    # One shared pool for scores AND projections: 3 double-bank buffers.
    # Sharing lifts the score buffers' reuse distance from 2 to >=3
    # allocations, breaking the exp->scores WAR chain that paces the kernel.
    ps_main = ctx.enter_context(tc.tile_pool(name="ps_main", bufs=3, space="PSUM"))
    ps_av = ctx.enter_context(tc.tile_pool(name="ps_av", bufs=2, space="PSUM"))
    expp = ctx.enter_context(tc.tile_pool(name="expp", bufs=8))
    recp = ctx.enter_context(tc.tile_pool(name="recp", bufs=6))
    stagep = ctx.enter_context(tc.tile_pool(name="stagep", bufs=6))
    outp = ctx.enter_context(tc.tile_pool(name="outp", bufs=4))

    # prime the ScalarE exp table set (~2.7us one-time load) during the
    # DMA phase instead of on the first real score tile
    warm = recp.tile([P, 1], F32, name="warm", tag="warm")
    nc.vector.memset(warm, 0.0)
    nc.scalar.activation(out=warm, in_=warm, func=EXP, scale=1.0)

    # ---- v projection (natural layout, no bias) ----------------------
    def emit_v_proj(tts=None):
        for tt in tts if tts is not None else range(KT):
            pst = ps_main.tile([P, 2 * QC], F32, name=f"vps{tt}", tag="sc")
            ps = pst[:, 0:D]
            for k in range(MT):
                nc.tensor.matmul(
                    ps,
                    lhsT=xT_sb[:, k, ts(tt, P)],
                    rhs=w_sb["wv"][:, k, :],
                    start=(k == 0),
                    stop=(k == MT - 1),
                )
            ps_h = ps.rearrange("p (h d) -> p h d", h=H)
            dst = v_sb[tt // 2][:, tt % 2]
            nc.vector.tensor_copy(out=dst[:, 0:H:2, 0:DH], in_=ps_h[:, 0:H:2, :])
            nc.vector.tensor_copy(out=dst[:, 1:H:2, DH:P], in_=ps_h[:, 1:H:2, :])

    # ---- q / k' projections for one feature tile m -------------------
    def emit_qk_proj(m, qcs=None):
        for qc in qcs if qcs is not None else range(NQC):
            pst = ps_main.tile([P, 2 * QC], F32, name=f"qk{m}_{qc}", tag="sc")
            ps = pst[:, 0:QC]
            for k in range(MT):
                nc.tensor.matmul(
                    ps,
                    lhsT=w_sb["wq"][:, k, ts(m, P)],
                    rhs=xT_sb[:, k, ts(qc, QC)],
                    start=(k == 0),
                    stop=(k == MT - 1),
                )
            nc.scalar.activation(
                out=qT_sb[m][:, ts(qc, QC)],
                in_=ps,
                func=IDENT,
                bias=b_sb["bq"][:, m : m + 1],
                scale=1.0,
            )
            ps2 = pst[:, QC : 2 * QC]
            for k in range(MT):
                nc.tensor.matmul(
                    ps2,
                    lhsT=w_sb["wk"][:, k, ts(m, P)],
                    rhs=xT_sb[:, k, ts(qc, QC)],
                    start=(k == 0),
                    stop=False,
                )
            for k in range(MT):
                nc.tensor.matmul(
                    ps2,
                    lhsT=w_sb["wp"][:, k, ts(m, P)],
                    rhs=posT_sb[:, k, ts(qc, QC)],
                    start=False,
                    stop=(k == MT - 1),
                )
            # the k-side bias (bk+bp) shifts each query's score row by a
            # constant, which softmax cancels -> plain copy, no bias.
            # Copy shares ScalarE's exp table set: no table switch cost.
            nc.scalar.activation(
                out=kT_sb[m][:, ts(qc, QC)], in_=ps2, func=COPY, scale=1.0
            )

    # ---- attention for head pair p (heads 2p, 2p+1) ------------------
    # The AV matmuls trail the score matmuls by one kt: TensorE's queue is
    # strict FIFO, so an AV emitted right after its exp head-of-line-blocks
    # PE for the exp's latency. Trailing by one kt keeps PE streaming
    # scores while the exp engines catch up.
    def emit_attention(p, on_qc_done=None, pre_kt=None):
        pend = []  # (qc, kt, psA, psB, e_sb) awaiting their AV matmuls
        norm_q = []  # (qc, psA, psB) awaiting normalization emission

        def emit_norm():
            qc, psA, psB = norm_q.pop(0)
            # fast evacuation: free the PSUM pair with two plain copies so
            # the next q-chunk's AV matmuls never head-of-line-block PE.
            # Split across ScalarE/DVE to balance the evacuation load.
            stA = stagep.tile([P, QC], F32, name=f"stA{p}_{qc}", tag="st")
            nc.scalar.activation(out=stA, in_=psA, func=COPY, scale=1.0)
            stB = stagep.tile([P, QC], F32, name=f"stB{p}_{qc}", tag="st")
            nc.scalar.activation(out=stB, in_=psB, func=COPY, scale=1.0)
            # stA = [ctxA @0:64 | denA @64:128]; stB = [denB @0:64 | ctxB @64:128]
            # gather both denominators into one tile -> single reciprocal
            den = recp.tile([P, QC], F32, name=f"den{p}_{qc}", tag="den")
            nc.sync.dma_start(out=den[0:DH], in_=stA[DH:P])  # cross to base 0
            nc.sync.dma_start(out=den[DH:P], in_=stB[0:DH])  # cross to base 64
            rec = recp.tile([P, QC], F32, name=f"rec{p}_{qc}", tag="rec")
            nc.vector.reciprocal_approx_fast(out=rec, in_=den)
            # normalization muls are pure SBUF work: offload to GpSimd,
            # except the final pair whose ctx gates the kernel tail.
            eng = nc.vector if p == MT - 1 else nc.gpsimd
            eng.tensor_mul(
                out=ctx_sb[p][0:DH, ts(qc, QC)], in0=stA[0:DH], in1=rec[0:DH]
            )
            eng.tensor_mul(
                out=ctx_sb[p][DH:P, ts(qc, QC)], in0=stB[DH:P], in1=rec[DH:P]
            )
            if on_qc_done is not None:
                on_qc_done(qc)

        def flush_av():
            if not pend:
                return
            fqc, fktp, fpsA, fpsB, fe = pend.pop(0)
            # attn @ V for the kt pair: 2 fp8 contraction rows per PE cell
            nc.tensor.matmul(
                fpsA,
                lhsT=v_sb[fktp][:, :, 2 * p, :],
                rhs=fe[:, :, 0:QC],
                start=(fktp == 0),
                stop=(fktp == KTP - 1),
                perf_mode=DR,
                skip_group_check=True,
            )
            nc.tensor.matmul(
                fpsB,
                lhsT=v_sb[fktp][:, :, 2 * p + 1, :],
                rhs=fe[:, :, QC : 2 * QC],
                start=(fktp == 0),
                stop=(fktp == KTP - 1),
                perf_mode=DR,
                skip_group_check=True,
            )
            if fktp == KTP - 1:
                emit_norm()

        for qc in range(NQC):
            psA = ps_av.tile([P, QC], F32, name=f"avA{p}_{qc}", tag="av")
            psB = ps_av.tile([P, QC], F32, name=f"avB{p}_{qc}", tag="av")
            norm_q.append((qc, psA, psB))
            for ktp in range(KTP):
                # e_pair[:, i, 0:QC] = head-A exps of kt=2*ktp+i; [:, i, QC:] = head B
                e_pair = expp.tile(
                    [P, 2, 2 * QC], FP8, name=f"e{p}_{qc}_{ktp}", tag="exp"
                )
                for i in range(2):
                    kt = 2 * ktp + i
                    if pre_kt is not None:
                        pre_kt(qc, kt)
                    s_ps = ps_main.tile(
                        [P, 2 * QC], F32, name=f"sc{p}_{qc}_{kt}", tag="sc"
                    )
                    sA, sB = s_ps[:, 0:QC], s_ps[:, QC : 2 * QC]
                    # scores^T = k'^T.T @ q^T, two heads packed via row tiling
                    nc.tensor.matmul(
                        sA,
                        lhsT=kT_sb[p][0:DH, ts(kt, P)],
                        rhs=qT_sb[p][0:DH, ts(qc, QC)],
                        start=True,
                        stop=True,
                    )
                    nc.tensor.matmul(
                        sB,
                        lhsT=kT_sb[p][DH:P, ts(kt, P)],
                        rhs=qT_sb[p][DH:P, ts(qc, QC)],
                        start=True,
                        stop=True,
                    )
                    e_out = e_pair[:, i, :]
                    if kt in SCHRAUDOLPH_KTS or (
                        p == MT - 1 and qc == NQC - 1 and kt >= KT - 2
                    ):
                        # last two exp tiles gate the whole tail; DVE's queue
                        # is empty by then while ScalarE still drains a backlog
                        nc.vector.tensor_scalar(
                            out=e_out.bitcast(U8),
                            in0=s_ps,
                            scalar1=_SCH_A,
                            scalar2=_SCH_B,
                            op0=MULT,
                            op1=ADD,
                        )
                    else:
                        nc.scalar.activation(
                            out=e_out, in_=s_ps, func=EXP, scale=0.125
                        )
                if len(pend) >= AV_TRAIL:
                    flush_av()
                pend.append((qc, ktp, psA, psB, e_pair))
            # qc boundary: drain one trailing AV now (its exps finished ktps
            # ago) so the normalization frees psA/psB a slot earlier and the
            # next q-chunk's first AV never head-of-line-blocks PE.
            flush_av()
        while pend:
            flush_av()

    # ---- output projection -------------------------------------------
    part_sb = {}

    def emit_out_proj_partial(qc):
        # contract pairs 0..2 for this q-chunk early (their ctx tiles are
        # done long before pair 3); the tail then needs only the k=3 matmul
        for m in range(MT):
            pst = ps_main.tile([P, 2 * QC], F32, name=f"pps{m}_{qc}", tag="sc")
            ps = pst[:, 0:QC]
            for k in range(MT - 1):
                nc.tensor.matmul(
                    ps,
                    lhsT=w_sb["wo"][:, k, ts(m, P)],
                    rhs=ctx_sb[k][:, ts(qc, QC)],
                    start=(k == 0),
                    stop=(k == MT - 2),
                )
            pt = outp.tile([P, QC], F32, name=f"part{m}_{qc}", tag="part")
            nc.scalar.activation(out=pt, in_=ps, func=COPY, scale=1.0)
            part_sb[m] = pt

    def emit_out_proj_final(qc):
        for m in range(MT):
            pst = ps_main.tile([P, 2 * QC], F32, name=f"ofin{m}_{qc}", tag="sc")
            ps = pst[:, 0:QC]
            nc.tensor.matmul(
                ps,
                lhsT=w_sb["wo"][:, MT - 1, ts(m, P)],
                rhs=ctx_sb[MT - 1][:, ts(qc, QC)],
                start=True,
                stop=True,
            )
            o_sb = outp.tile([P, QC], F32, name=f"of{m}_{qc}", tag="out")
            nc.vector.scalar_tensor_tensor(
                out=o_sb,
                in0=ps,
                scalar=b_sb["bob"][:, m : m + 1],
                in1=part_sb[m],
                op0=ADD,
                op1=ADD,
            )
            nc.sync.dma_start(out=io["outT"][m][:, ts(qc, QC)], in_=o_sb)

    def emit_out_proj(qc):
        for m in range(MT):
            pst = ps_main.tile([P, 2 * QC], F32, name=f"ops{m}_{qc}", tag="sc")
            ps = pst[:, 0:QC]
            for k in range(MT):
                nc.tensor.matmul(
                    ps,
                    lhsT=w_sb["wo"][:, k, ts(m, P)],
                    rhs=ctx_sb[k][:, ts(qc, QC)],
                    start=(k == 0),
                    stop=(k == MT - 1),
                )
            o_sb = outp.tile([P, QC], F32, name=f"o{m}_{qc}", tag="out")
            nc.scalar.activation(
                out=o_sb,
                in_=ps,
                func=IDENT,
                bias=b_sb["bob"][:, m : m + 1],
                scale=1.0,
            )
            nc.sync.dma_start(out=io["outT"][m][:, ts(qc, QC)], in_=o_sb)

    # emission order: attention (ScalarE-bound) starts as early as possible;
    # remaining projections backfill TensorE while ScalarE streams exps.
    emit_qk_proj(0, qcs=[0])
    emit_v_proj(tts=range(0, 2))

    def p0_hook(qc, kt):
        # pair-0 runs while inputs still stream in: emit the remaining
        # projections just-in-time so early scores/exps aren't scheduled
        # behind load-gated work.
        if qc != 0:
            return
        tt = kt + 2
        if tt < KT:
            emit_v_proj(tts=[tt])
        if kt == 1:
            emit_qk_proj(0, qcs=[1])
        if kt == 5:
            emit_qk_proj(0, qcs=[2])
        if kt == 9:
            emit_qk_proj(0, qcs=[3])

    emit_attention(0, pre_kt=p0_hook)
    for m in range(1, MT - 1):
        emit_qk_proj(m)
        emit_attention(m)
    emit_qk_proj(MT - 1)

    def out_proj_lagged(qc):
        if qc > 0:
            emit_out_proj(qc - 1)

    def p3_pre(qc, kt):
        # pre-contract pairs 0..2 of the final q-chunk mid-loop so the tail
        # only needs the k=3 matmuls after pair 3's normalization lands
        if qc == NQC - 1 and kt == 10:
            emit_out_proj_partial(qc)

    emit_attention(MT - 1, on_qc_done=out_proj_lagged, pre_kt=p3_pre)
    emit_out_proj_final(NQC - 1)

    if dbg is not None:
        for m in range(MT):
            nc.sync.dma_start(out=dbg["qT"][m], in_=qT_sb[m])
            nc.sync.dma_start(out=dbg["kT"][m], in_=kT_sb[m])
            nc.sync.dma_start(out=dbg["ctxT"][m], in_=ctx_sb[m])
        nc.sync.dma_start(out=dbg["v0"], in_=v_sb[0])


_CACHED_NC = None


def build_nc():
    global _CACHED_NC
    if _CACHED_NC is not None:
        return _CACHED_NC
    nc = bacc.Bacc("TRN2", target_bir_lowering=False, debug=False, num_devices=N_CORES)
    io = {}
    # partition-major layouts so each tensor is a single large DMA
    io["xT"] = nc.dram_tensor("xT", [P, MT, T], BF, kind="ExternalInput").ap()
    io["posT"] = nc.dram_tensor("posT", [P, MT, T], BF, kind="ExternalInput").ap()
    for wname in ("wq", "wk", "wp", "wv", "wo"):
        io[wname + "T"] = nc.dram_tensor(
            wname + "T", [P, MT, D], BF, kind="ExternalInput"
        ).ap()
    for bname in ("bq", "bob"):
        io[bname] = nc.dram_tensor(bname, [P, MT], F32, kind="ExternalInput").ap()
    io["outT"] = nc.dram_tensor("outT", [MT, P, T], F32, kind="ExternalOutput").ap()

    from contextlib import ExitStack

    with tile.TileContext(nc) as tc, ExitStack() as ctx:
        _emit(ctx, tc, io)
    nc.compile()
    _CACHED_NC = nc
    return nc


def _to_bf16(a):
    return np.asarray(a, dtype=np.float32).astype(ml_dtypes.bfloat16)


def _pmajor_w(W):
    # [d_out, d_in] -> [P, MT, D]: partition p, k-tile k holds W.T[k*P+p, :]
    return _to_bf16(
        np.asarray(W, np.float32).T.reshape(MT, P, D).transpose(1, 0, 2)
    )


def make_in_maps(x, pos_embeddings, Wq, bq, Wk, bk, Wv, bv, Wp, bp, Wo, bo):
    """Host-side prep: transpose / retile / fold biases / cast to bf16."""
    x = np.asarray(x, np.float32)
    pos = np.asarray(pos_embeddings, np.float32)
    wqT = _pmajor_w(Wq)
    wkT = _pmajor_w(Wk)
    wpT = _pmajor_w(Wp)
    wvT = _pmajor_w(Wv)
    woT = _pmajor_w(Wo)
    bq_t = np.ascontiguousarray(np.asarray(bq, np.float32).reshape(MT, P).T)
    bob = np.ascontiguousarray(
        (
            np.asarray(bo, np.float32)
            + np.asarray(Wo, np.float32) @ np.asarray(bv, np.float32)
        ).reshape(MT, P).T
    )

    in_maps = []
    for b in range(B):
        xT = _to_bf16(
            np.ascontiguousarray(x[b].T).reshape(MT, P, T).transpose(1, 0, 2)
        )
        posT = _to_bf16(
            np.ascontiguousarray(pos[b].T).reshape(MT, P, T).transpose(1, 0, 2)
        )
        in_maps.append(
            dict(
                xT=xT,
                posT=posT,
                wqT=wqT,
                wkT=wkT,
                wpT=wpT,
                wvT=wvT,
                woT=woT,
                bq=bq_t,
                bob=bob,
            )
        )
    return in_maps


def assemble_output(results):
    out = np.empty((B, T, D), np.float32)
    for b in range(B):
        out[b] = results[b]["outT"].reshape(D, T).T
    return out


def kernel(**inputs) -> np.ndarray:
    nc = build_nc()
    in_maps = make_in_maps(**inputs)
    res = run_bass_kernel_spmd(nc, in_maps, core_ids=list(range(N_CORES)))
    return assemble_output(res.results)


if __name__ == "__main__":
    import reference

    inputs = {k: np.asarray(v) for k, v in reference.setup_inputs().items()}
    got = kernel(**inputs)
    exp = np.asarray(reference.reference(**inputs))
    err = np.abs(got - exp)
    rel = np.linalg.norm(got - exp) / np.linalg.norm(exp)
    print("max abs err:", err.max(), "rel:", rel)

